# revision 7
# baseline (speedup 1.0000x reference)
"""Multi-head attention (B=2, S=2048, D=1024, H=16) on 8 Trainium2 NeuronCores.

Sharding: batch x head-group. Core c handles batch b = c//4 and heads
[4*(c%4), 4*(c%4)+4) (a 256-wide slice of the QKV projection output and the
matching 256-row slice of Wo). Each core computes its partial output
projection; a 4-way ReduceScatter per batch group sums the partials and
leaves each core with a [512, 1024] row block of the final output, which the
host reassembles.

v2 dataflow (all matmul operands fp16, fp32 PSUM accumulation):
  - x^T tiles via hardware DMA-transpose, token-split halves across BOTH
    HWDGE queues (SP + Activation) with cross-queue copy<->transpose
    ordering edges for the xbar-mode hazard.
  - EVERY matmul is a 64-out-row col-tiled pair (tile_position (0,0)/(0,64))
    in one uniform PE tiling mode: pairs dual-issue on the PE array halves
    (observed ~263ns per 512-col pair vs ~330ns for a plain 128-row matmul)
    and the constant mode means no array drains anywhere.
  - Attention is software-pipelined at (slot = (q-chunk, head-pair), kp =
    2-k-tile group) granularity: scores(s) | exp(s) on Scalar | attn@V+sums
    of slot s-1 | projection / output-projection fillers, sized so the
    Scalar engine's exp stream (the ~129us serial floor) stays saturated.
  - Softmax without max-subtraction; per-q sums via ones-matmuls col-packed
    with the attn@V pairs; normalization multiply on DVE; proj biases on DVE
    (tensor_scalar) keeping Scalar exp-only.
"""

import numpy as np

import concourse.bass as bass  # noqa: F401  (engine namespaces via nc)
import concourse.mybir as mybir
import concourse.tile as tile
from concourse import bacc
from concourse.bass import _add_dep_helper
from concourse.bass_utils import run_bass_kernel_spmd

F32 = mybir.dt.float32
F16 = mybir.dt.float16
AF = mybir.ActivationFunctionType

B, S, D = 2, 2048, 1024
H, DH = 16, 64
NCORES = 8
GPB = 4                # cores per batch group
HPC = H // GPB         # heads per core
DS = HPC * DH          # 256: per-core slice of the projection output
P = 128
NDT = D // P           # 8 d_model tiles
NTT = S // P           # 16 token tiles
QCH = 512              # q-chunk (PSUM bank = 512 fp32)
NQC = S // QCH         # 4
NKT = S // P           # 16 k tiles
HS = S // 2            # token-split half for the DMA transposes
SCALE = float(1.0 / np.sqrt(DH))

REPLICA_GROUPS = [[0, 1, 2, 3], [4, 5, 6, 7]]

_CACHED_NC = None


def _build_module():
    nc = bacc.Bacc("TRN2", target_bir_lowering=False, debug=False,
                   num_devices=NCORES)

    xq_d = nc.dram_tensor("xq", [S, D], F16, kind="ExternalInput")
    xk_d = nc.dram_tensor("xk", [S, D], F16, kind="ExternalInput")
    xv_d = nc.dram_tensor("xv", [S, D], F16, kind="ExternalInput")
    wq_d = nc.dram_tensor("wq", [D, DS], F16, kind="ExternalInput")
    wk_d = nc.dram_tensor("wk", [D, DS], F16, kind="ExternalInput")
    wv_d = nc.dram_tensor("wv", [D, DS], F16, kind="ExternalInput")
    wo_d = nc.dram_tensor("wo", [DS, D], F16, kind="ExternalInput")
    bq_d = nc.dram_tensor("bq", [DS, 1], F32, kind="ExternalInput")
    bk_d = nc.dram_tensor("bk", [DS, 1], F32, kind="ExternalInput")
    bv_d = nc.dram_tensor("bv", [1, DS], F32, kind="ExternalInput")
    bo_d = nc.dram_tensor("bo", [1, D], F32, kind="ExternalInput")

    out_d = nc.dram_tensor("out", [S // GPB, D], F16, kind="ExternalOutput")
    partial_cs = [nc.dram_tensor(f"partial{j}", [4 * P, D], F16)
                  for j in range(4)]
    rs_cs = [nc.dram_tensor(f"rs_out{j}", [P, D], F16)
             for j in range(4)]

    with tile.TileContext(nc) as tc:
        with (
            tc.tile_pool(name="cst", bufs=1) as cst,
            tc.tile_pool(name="exp", bufs=16) as expp,
            tc.tile_pool(name="rcp", bufs=2) as rcpp,
            tc.tile_pool(name="osb", bufs=3) as osbp,
            tc.tile_pool(name="ps", bufs=2, space="PSUM") as psp,
            tc.tile_pool(name="sm", bufs=2, space="PSUM") as smp,
            tc.tile_pool(name="acc", bufs=2, space="PSUM") as accp,
        ):
            # One uniform PE tiling mode (128-contraction, 64-out-row col
            # pairs) -> never a mode flip; nosync chain just pins emission
            # order so the interleave survives scheduling.
            _real_matmul = nc.tensor.matmul
            _prev_mm = {"inst": None}

            def mm(out, lhsT, rhs, **kw):
                inst = _real_matmul(out, lhsT, rhs, skip_group_check=True,
                                    **kw)
                if _prev_mm["inst"] is not None:
                    _add_dep_helper(inst.ins, _prev_mm["inst"].ins,
                                    sync=False, reason="pe-order")
                _prev_mm["inst"] = inst
                return inst

            def mm_pair(out, lhsT_ap, rhs, col0, **kw):
                """Two 64-row col-tiled matmuls writing out[0:64]/[64:128].

                lhsT_ap: callable half -> AP of [128, 64] weights for that
                output-row half; rhs shared.
                """
                mm(out[0:64, :], lhsT_ap(0), rhs,
                   tile_position=(0, 0), **kw)
                mm(out[64:128, :], lhsT_ap(1), rhs,
                   tile_position=(0, 64), **kw)

            # ---- constants ----
            wq_t = cst.tile([P, NDT, DS], F16, tag="wq")
            wk_t = cst.tile([P, NDT, DS], F16, tag="wk")
            wv_t = cst.tile([P, NDT, DS], F16, tag="wv")
            wo_t = cst.tile([P, 2, D], F16, tag="wo")
            bq_t = cst.tile([P, 2, 1], F32, tag="bq")
            bk_t = cst.tile([P, 2, 1], F32, tag="bk")
            bv_row = cst.tile([1, DS], F32, tag="bvr")
            bo_row = cst.tile([1, D], F32, tag="bor")
            bv_b = cst.tile([P, DS], F32, tag="bvb")
            bo_b = cst.tile([P, D], F32, tag="bob")
            ones_t = cst.tile([P, DH], F16, tag="ones")

            # ---- activations: resident tensors ----
            qt_t = cst.tile([P, 2, S], F16, tag="qt")    # Q^T  (pair, t)
            kz_t = cst.tile([P, HPC, S], F16, tag="kz")  # zero-padded K^T
            v_t = cst.tile([P, NTT, DS], F16, tag="vt")  # V token-major
            an_t = cst.tile([P, 2, S], F16, tag="an")    # attn_norm^T
            xt_k = cst.tile([P, NDT, S], F16, tag="xtk")
            xt_q = cst.tile([P, NDT, S], F16, tag="xtq")
            xt_v = cst.tile([P, NDT, S], F16, tag="xtv")

            # ---- DMA: one queue (SP), FIFO pinned with chain deps ----
            # The DMA xbar has a global transpose/copy mode and ~fixed
            # aggregate transpose throughput: concurrent queues just contend
            # (measured 0.65x each) and any copy in flight during transpose
            # mode corrupts (the v2 failure: the scheduler hoisted wv/wo
            # between transposes on the other queue). So: one queue, copies
            # strictly before transposes, order pinned.
            _prev_dma = {"inst": None}

            def dma(out, in_, **kw):
                inst = nc.sync.dma_start(out, in_, **kw)
                if _prev_dma["inst"] is not None:
                    _add_dep_helper(inst.ins, _prev_dma["inst"].ins,
                                    sync=False, reason="dma-fifo")
                _prev_dma["inst"] = inst
                return inst

            dma(wk_t[:], wk_d.rearrange("(a p) n -> p a n", p=P))
            dma(bk_t[:], bk_d.rearrange("(a p) o -> p a o", p=P))
            dma(bq_t[:], bq_d.rearrange("(a p) o -> p a o", p=P))
            dma(wq_t[:], wq_d.rearrange("(a p) n -> p a n", p=P))
            dma(wv_t[:], wv_d.rearrange("(a p) n -> p a n", p=P))
            dma(bv_row[:], bv_d[:])
            # order: K fully, then Q's first half (q-chunk 0/1), then V
            # (needed by slot-0 attn@V fillers), then Q's second half
            # (first needed by scores slot 2, ~66us in)
            dma(xt_k[:, :, 0:HS], xk_d[0:HS, :], transpose=True)
            dma(xt_k[:, :, HS:S], xk_d[HS:S, :], transpose=True)
            dma(xt_q[:, :, 0:HS], xq_d[0:HS, :], transpose=True)
            dma(xt_v[:, :, 0:HS], xv_d[0:HS, :], transpose=True)
            dma(xt_v[:, :, HS:S], xv_d[HS:S, :], transpose=True)
            dma(xt_q[:, :, HS:S], xq_d[HS:S, :], transpose=True)
            dma(wo_t[:], wo_d.rearrange("(a p) n -> p a n", p=P))
            dma(bo_row[:], bo_d[:])
            nc.gpsimd.partition_broadcast(bv_b[:], bv_row[:])
            nc.gpsimd.partition_broadcast(bo_b[:], bo_row[:])

            nc.vector.memset(kz_t[:], 0.0)
            nc.vector.memset(ones_t[:], 1.0)

            # ---- quanta ----
            def q_kproj(tc_idx):
                ts0 = tc_idx * QCH
                ps = psp.tile([P, 2 * QCH], F32, tag="sc")
                for dot in range(2):
                    col = slice(dot * QCH, (dot + 1) * QCH)
                    for dt in range(NDT):
                        mm_pair(
                            ps[:, col],
                            lambda h, dt=dt, dot=dot: wk_t[
                                :, dt, dot * P + h * 64:dot * P + (h + 1) * 64],
                            xt_k[:, dt, ts0:ts0 + QCH],
                            None,
                            start=(dt == 0), stop=(dt == NDT - 1))
                # per-head 64-row slices into the padded K^T; bias on DVE
                for h in range(HPC):
                    rows = slice((h % 2) * 64, (h % 2) * 64 + 64)
                    dot = h // 2
                    nc.vector.tensor_scalar_add(
                        kz_t[rows, h, ts0:ts0 + QCH],
                        ps[rows, dot * QCH:(dot + 1) * QCH],
                        bk_t[rows, dot, :])

            def q_qproj(tc_idx):
                ts0 = tc_idx * QCH
                ps = psp.tile([P, 2 * QCH], F32, tag="sc")
                for dot in range(2):
                    col = slice(dot * QCH, (dot + 1) * QCH)
                    for dt in range(NDT):
                        mm_pair(
                            ps[:, col],
                            lambda h, dt=dt, dot=dot: wq_t[
                                :, dt, dot * P + h * 64:dot * P + (h + 1) * 64],
                            xt_q[:, dt, ts0:ts0 + QCH],
                            None,
                            start=(dt == 0), stop=(dt == NDT - 1))
                for dot in range(2):
                    nc.vector.tensor_scalar_add(
                        qt_t[:, dot, ts0:ts0 + QCH],
                        ps[:, dot * QCH:(dot + 1) * QCH],
                        bq_t[:, dot, :])

            def q_vproj(tt):
                ps = psp.tile([P, 2 * QCH], F32, tag="sc")
                for dt in range(NDT):
                    mm_pair(
                        ps[:, 0:DS],
                        lambda h, dt=dt, tt=tt: xt_v[
                            :, dt, tt * P + h * 64:tt * P + (h + 1) * 64],
                        wv_t[:, dt, :],
                        None,
                        start=(dt == 0), stop=(dt == NDT - 1))
                nc.vector.tensor_add(v_t[:, tt, :], ps[:, 0:DS], bv_b[:, :])

            e_ring = {}

            def q_scores(s, kp):
                qc, pr = divmod(s, 2)
                qs = qc * QCH
                pair = []
                for hh in range(2):
                    hsel = 2 * pr + hh
                    sc = psp.tile([P, 2 * QCH], F32, tag="sc")
                    for j in range(2):
                        ks = (2 * kp + j) * P
                        col = slice(j * QCH, (j + 1) * QCH)
                        mm_pair(
                            sc[:, col],
                            lambda h, hsel=hsel, ks=ks: kz_t[
                                :, hsel, ks + h * 64:ks + (h + 1) * 64],
                            qt_t[:, pr, qs:qs + QCH],
                            None,
                            start=True, stop=True)
                    e = expp.tile([P, 2 * QCH], F16, tag="exp")
                    nc.scalar.activation(e[:], sc[:], AF.Exp, scale=SCALE)
                    pair.append(e)
                e_ring[(s, kp)] = pair

            acc_sm = {}

            def q_ph2(s, kp):
                qc, pr = divmod(s, 2)
                h0 = 2 * pr
                h1 = 2 * pr + 1
                if s not in acc_sm:
                    acc_sm[s] = (accp.tile([P, QCH], F32, tag="acc", name="acc"),
                                 smp.tile([P, QCH], F32, tag="sum", name="sm"))
                acc, sm = acc_sm[s]
                e0, e1 = e_ring.pop((s, kp))
                for j in range(2):
                    kt = 2 * kp + j
                    col = slice(j * QCH, (j + 1) * QCH)
                    st = (kt == 0)
                    sp = (kt == NKT - 1)
                    mm(sm[0:64, :], ones_t[:], e0[:, col],
                       start=st, stop=sp, tile_position=(0, 0))
                    mm(sm[64:128, :], ones_t[:], e1[:, col],
                       start=st, stop=sp, tile_position=(0, 64))
                    mm(acc[0:64, :], v_t[:, kt, h0 * DH:(h0 + 1) * DH],
                       e0[:, col], start=st, stop=sp, tile_position=(0, 0))
                    mm(acc[64:128, :], v_t[:, kt, h1 * DH:(h1 + 1) * DH],
                       e1[:, col], start=st, stop=sp, tile_position=(0, 64))

            def q_ph2_end(s):
                qc, pr = divmod(s, 2)
                qs = qc * QCH
                acc, sm = acc_sm.pop(s)
                rc = rcpp.tile([P, QCH], F32, tag="rcp")
                nc.vector.reciprocal_approx_fast(rc[:], sm[:])
                nc.vector.tensor_mul(an_t[:, pr, qs:qs + QCH], acc[:], rc[:])

            def q_outproj(qc, tt4):
                tt = qc * 4 + tt4
                po = psp.tile([P, 2 * QCH], F32, tag="sc")
                for half in range(2):
                    col = slice(half * QCH, (half + 1) * QCH)
                    for pr in range(2):
                        mm_pair(
                            po[:, col],
                            lambda h, pr=pr, tt=tt: an_t[
                                :, pr, tt * P + h * 64:tt * P + (h + 1) * 64],
                            wo_t[:, pr, half * QCH:(half + 1) * QCH],
                            None,
                            start=(pr == 0), stop=(pr == 1))
                ob = osbp.tile([P, D], F16, tag="osb")
                nc.vector.tensor_add(ob[:], po[:], bo_b[:])
                nc.sync.dma_start(
                    partial_cs[tt // 4][(tt % 4) * P:(tt % 4 + 1) * P, :],
                    ob[:])

            def q_rs(qc):
                nc.gpsimd.collective_compute(
                    "ReduceScatter",
                    mybir.AluOpType.add,
                    replica_groups=REPLICA_GROUPS,
                    ins=[partial_cs[qc][:]],
                    outs=[rs_cs[qc][:]],
                )
                nc.sync.dma_start(out_d[qc * P:(qc + 1) * P, :],
                                  rs_cs[qc][:])

            # ---- schedule ----
            for tci in range(NQC):
                q_kproj(tci)
            q_qproj(0)

            # slot 0: scores + all of the V projection
            for kp in range(8):
                q_scores(0, kp)
                q_vproj(2 * kp)
                q_vproj(2 * kp + 1)

            # slots 1..7: steady pipeline, ph2 lags scores by one slot
            for s in range(1, 8):
                for kp in range(8):
                    q_scores(s, kp)
                    q_ph2(s - 1, kp)
                    if s in (1, 2, 4) and kp == 4:
                        q_qproj({1: 1, 2: 2, 4: 3}[s])
                    if s in (3, 5, 7) and kp % 2 == 1:
                        q_outproj({3: 0, 5: 1, 7: 2}[s], kp // 2)
                    if s == 7 and kp >= 2:
                        q_ph2(7, kp - 2)
                q_ph2_end(s - 1)
                if s == 3:
                    q_rs(0)
                if s == 5:
                    q_rs(1)
            q_rs(2)

            # tail
            q_ph2(7, 6)
            q_ph2(7, 7)
            q_ph2_end(7)
            for tt4 in range(4):
                q_outproj(3, tt4)
            q_rs(3)

    nc.compile()
    return nc


def _get_nc():
    global _CACHED_NC
    if _CACHED_NC is None:
        _CACHED_NC = _build_module()
    return _CACHED_NC


def _swap_pairs_rows(wo_slice):
    """Swap the two 64-row head blocks within each head pair (phase-2 PSUM
    layout has the pair's heads in partitions 0-63 / 64-127)."""
    out = wo_slice.copy()
    for pr in range(2):
        a = pr * 2 * DH
        out[a:a + DH], out[a + DH:a + 2 * DH] = \
            wo_slice[a + DH:a + 2 * DH].copy(), wo_slice[a:a + DH].copy()
    return out


def _make_in_maps(query, key, value, Wq, bq, Wk, bk, Wv, bv, Wo, bo):
    query = np.asarray(query, dtype=np.float32)
    key = np.asarray(key, dtype=np.float32)
    value = np.asarray(value, dtype=np.float32)
    Wq = np.asarray(Wq, dtype=np.float32)
    Wk = np.asarray(Wk, dtype=np.float32)
    Wv = np.asarray(Wv, dtype=np.float32)
    Wo = np.asarray(Wo, dtype=np.float32)
    bq = np.asarray(bq, dtype=np.float32)
    bk = np.asarray(bk, dtype=np.float32)
    bv = np.asarray(bv, dtype=np.float32)
    bo = np.asarray(bo, dtype=np.float32)

    in_maps = []
    for c in range(NCORES):
        b = c // GPB
        g = c % GPB
        sl = slice(g * DS, (g + 1) * DS)
        in_maps.append({
            "xq": query[b].astype(np.float16),
            "xk": key[b].astype(np.float16),
            "xv": value[b].astype(np.float16),
            "wq": Wq[:, sl].astype(np.float16),
            "wk": Wk[:, sl].astype(np.float16),
            "wv": Wv[:, sl].astype(np.float16),
            "wo": Wo[sl, :].astype(np.float16),
            "bq": bq[sl].reshape(DS, 1).copy(),
            "bk": bk[sl].reshape(DS, 1).copy(),
            "bv": bv[sl].reshape(1, DS).copy(),
            "bo": (bo if g == 0 else np.zeros_like(bo)).reshape(1, D).copy(),
        })
    return in_maps


def run(inputs, trace=False, trace_cores=None):
    """Run the SPMD kernel; returns (full_output, BassKernelResults)."""
    nc = _get_nc()
    in_maps = _make_in_maps(**inputs)
    res = run_bass_kernel_spmd(
        nc, in_maps, core_ids=list(range(NCORES)), trace=trace,
        trace_cores=trace_cores)
    out = np.empty((B, S, D), dtype=np.float32)
    for c in range(NCORES):
        b = c // GPB
        g = c % GPB
        o = res.results[c]["out"].astype(np.float32)
        for j in range(4):
            out[b, j * 512 + g * P:j * 512 + (g + 1) * P, :] = \
                o[j * P:(j + 1) * P, :]
    return out, res


def kernel(**inputs):
    out, _ = run(inputs, trace=False)
    return out


# revision 9
# speedup vs baseline: 1.2567x; 1.2567x over previous
"""Multi-head attention (B=2, S=2048, D=1024, H=16) on 8 Trainium2 NeuronCores.

Sharding: batch x head-group. Core c handles batch b = c//4 and heads
[4*(c%4), 4*(c%4)+4) (a 256-wide slice of the QKV projection output and the
matching 256-row slice of Wo). Each core computes its partial output
projection; a 4-way ReduceScatter per batch group sums the partials and
leaves each core with a [512, 1024] row block of the final output, which the
host reassembles.

v2 dataflow (all matmul operands fp16, fp32 PSUM accumulation):
  - x^T tiles via hardware DMA-transpose, token-split halves across BOTH
    HWDGE queues (SP + Activation) with cross-queue copy<->transpose
    ordering edges for the xbar-mode hazard.
  - EVERY matmul is a 64-out-row col-tiled pair (tile_position (0,0)/(0,64))
    in one uniform PE tiling mode: pairs dual-issue on the PE array halves
    (observed ~263ns per 512-col pair vs ~330ns for a plain 128-row matmul)
    and the constant mode means no array drains anywhere.
  - Attention is software-pipelined at (slot = (q-chunk, head-pair), kp =
    2-k-tile group) granularity: scores(s) | exp(s) on Scalar | attn@V+sums
    of slot s-1 | projection / output-projection fillers, sized so the
    Scalar engine's exp stream (the ~129us serial floor) stays saturated.
  - Softmax without max-subtraction; per-q sums via ones-matmuls col-packed
    with the attn@V pairs; normalization multiply on DVE; proj biases on DVE
    (tensor_scalar) keeping Scalar exp-only.
"""

import numpy as np

import concourse.bass as bass  # noqa: F401  (engine namespaces via nc)
import concourse.mybir as mybir
import concourse.tile as tile
from concourse import bacc
from concourse.bass import _add_dep_helper
from concourse.bass_utils import run_bass_kernel_spmd

F32 = mybir.dt.float32
F16 = mybir.dt.float16
AF = mybir.ActivationFunctionType

B, S, D = 2, 2048, 1024
H, DH = 16, 64
NCORES = 8
GPB = 4                # cores per batch group
HPC = H // GPB         # heads per core
DS = HPC * DH          # 256: per-core slice of the projection output
P = 128
NDT = D // P           # 8 d_model tiles
NTT = S // P           # 16 token tiles
QCH = 512              # q-chunk (PSUM bank = 512 fp32)
NQC = S // QCH         # 4
NKT = S // P           # 16 k tiles
HS = S // 2            # token-split half for the DMA transposes
SCALE = float(1.0 / np.sqrt(DH))

REPLICA_GROUPS = [[0, 1, 2, 3], [4, 5, 6, 7]]

_CACHED_NC = None


def _build_module():
    nc = bacc.Bacc("TRN2", target_bir_lowering=False, debug=False,
                   num_devices=NCORES)

    xq_d = nc.dram_tensor("xq", [S, D], F16, kind="ExternalInput")
    xk_d = nc.dram_tensor("xk", [S, D], F16, kind="ExternalInput")
    xv_d = nc.dram_tensor("xv", [S, D], F16, kind="ExternalInput")
    wq_d = nc.dram_tensor("wq", [D, DS], F16, kind="ExternalInput")
    wk_d = nc.dram_tensor("wk", [D, DS], F16, kind="ExternalInput")
    wv_d = nc.dram_tensor("wv", [D, DS], F16, kind="ExternalInput")
    wo_d = nc.dram_tensor("wo", [DS, D], F16, kind="ExternalInput")
    bq_d = nc.dram_tensor("bq", [DS, 1], F32, kind="ExternalInput")
    bk_d = nc.dram_tensor("bk", [DS, 1], F32, kind="ExternalInput")
    bv_d = nc.dram_tensor("bv", [1, DS], F32, kind="ExternalInput")
    bo_d = nc.dram_tensor("bo", [1, D], F32, kind="ExternalInput")

    out_d = nc.dram_tensor("out", [S // GPB, D], F16, kind="ExternalOutput")
    partial_cs = [nc.dram_tensor(f"partial{j}", [4 * P, D], F16)
                  for j in range(4)]
    rs_cs = [nc.dram_tensor(f"rs_out{j}", [P, D], F16)
             for j in range(4)]

    with tile.TileContext(nc) as tc:
        with (
            tc.tile_pool(name="cst", bufs=1) as cst,
            tc.tile_pool(name="exp", bufs=16) as expp,
            tc.tile_pool(name="rcp", bufs=2) as rcpp,
            tc.tile_pool(name="osb", bufs=3) as osbp,
            tc.tile_pool(name="ps", bufs=2, space="PSUM") as psp,
            tc.tile_pool(name="sm", bufs=2, space="PSUM") as smp,
            tc.tile_pool(name="acc", bufs=2, space="PSUM") as accp,
        ):
            # One uniform PE tiling mode (128-contraction, 64-out-row col
            # pairs) -> never a mode flip; nosync chain just pins emission
            # order so the interleave survives scheduling.
            _real_matmul = nc.tensor.matmul
            _prev_mm = {"inst": None}

            def mm(out, lhsT, rhs, **kw):
                inst = _real_matmul(out, lhsT, rhs, skip_group_check=True,
                                    **kw)
                if _prev_mm["inst"] is not None:
                    _add_dep_helper(inst.ins, _prev_mm["inst"].ins,
                                    sync=False, reason="pe-order")
                _prev_mm["inst"] = inst
                return inst

            def mm_pair(out, lhsT_ap, rhs, col0, **kw):
                """Two 64-row col-tiled matmuls writing out[0:64]/[64:128].

                lhsT_ap: callable half -> AP of [128, 64] weights for that
                output-row half; rhs shared.
                """
                mm(out[0:64, :], lhsT_ap(0), rhs,
                   tile_position=(0, 0), **kw)
                mm(out[64:128, :], lhsT_ap(1), rhs,
                   tile_position=(0, 64), **kw)

            # ---- constants ----
            wq_t = cst.tile([P, NDT, DS], F16, tag="wq")
            wk_t = cst.tile([P, NDT, DS], F16, tag="wk")
            wv_t = cst.tile([P, NDT, DS], F16, tag="wv")
            wo_t = cst.tile([P, 2, D], F16, tag="wo")
            bq_t = cst.tile([P, 2, 1], F32, tag="bq")
            bk_t = cst.tile([P, 2, 1], F32, tag="bk")
            bv_row = cst.tile([1, DS], F32, tag="bvr")
            bo_row = cst.tile([1, D], F32, tag="bor")
            bv_b = cst.tile([P, DS], F32, tag="bvb")
            bo_b = cst.tile([P, D], F32, tag="bob")
            ones_t = cst.tile([P, DH], F16, tag="ones")

            # ---- activations: resident tensors ----
            qt_t = cst.tile([P, 2, S], F16, tag="qt")    # Q^T  (pair, t)
            kz_t = cst.tile([P, HPC, S], F16, tag="kz")  # zero-padded K^T
            v_t = cst.tile([P, NTT, DS], F16, tag="vt")  # V token-major
            an_t = cst.tile([P, 2, S], F16, tag="an")    # attn_norm^T
            xt_k = cst.tile([P, NDT, S], F16, tag="xtk")
            xt_q = cst.tile([P, NDT, S], F16, tag="xtq")
            xt_v = cst.tile([P, NDT, S], F16, tag="xtv")

            # ---- DMA: one queue (SP), FIFO pinned with chain deps ----
            # The DMA xbar has a global transpose/copy mode and ~fixed
            # aggregate transpose throughput: concurrent queues just contend
            # (measured 0.65x each) and any copy in flight during transpose
            # mode corrupts (the v2 failure: the scheduler hoisted wv/wo
            # between transposes on the other queue). So: one queue, copies
            # strictly before transposes, order pinned.
            _prev_dma = {"inst": None}

            def dma(out, in_, **kw):
                inst = nc.sync.dma_start(out, in_, **kw)
                if _prev_dma["inst"] is not None:
                    _add_dep_helper(inst.ins, _prev_dma["inst"].ins,
                                    sync=False, reason="dma-fifo")
                _prev_dma["inst"] = inst
                return inst

            # Order tuned for earliest exp-start: K first (gates everything),
            # Q in quarters (scores slot s needs only q-chunk s//2), V halves
            # next (slot-0/1 attn@V), remaining copies slotted between.
            dma(wk_t[:], wk_d.rearrange("(a p) n -> p a n", p=P))
            dma(bk_t[:], bk_d.rearrange("(a p) o -> p a o", p=P))
            dma(xt_k[:, :, 0:HS], xk_d[0:HS, :], transpose=True)
            dma(xt_k[:, :, HS:S], xk_d[HS:S, :], transpose=True)
            dma(xt_q[:, :, 0:QCH], xq_d[0:QCH, :], transpose=True)
            dma(bq_t[:], bq_d.rearrange("(a p) o -> p a o", p=P))
            dma(wq_t[:], wq_d.rearrange("(a p) n -> p a n", p=P))
            dma(wv_t[:], wv_d.rearrange("(a p) n -> p a n", p=P))
            dma(bv_row[:], bv_d[:])
            dma(xt_v[:, :, 0:HS], xv_d[0:HS, :], transpose=True)
            dma(xt_v[:, :, HS:S], xv_d[HS:S, :], transpose=True)
            dma(xt_q[:, :, QCH:2 * QCH], xq_d[QCH:2 * QCH, :],
                transpose=True)
            dma(wo_t[:], wo_d.rearrange("(a p) n -> p a n", p=P))
            dma(bo_row[:], bo_d[:])
            dma(xt_q[:, :, 2 * QCH:3 * QCH], xq_d[2 * QCH:3 * QCH, :],
                transpose=True)
            dma(xt_q[:, :, 3 * QCH:S], xq_d[3 * QCH:S, :], transpose=True)
            nc.gpsimd.partition_broadcast(bv_b[:], bv_row[:])
            nc.gpsimd.partition_broadcast(bo_b[:], bo_row[:])

            nc.vector.memset(kz_t[:], 0.0)
            nc.vector.memset(ones_t[:], 1.0)

            # ---- quanta ----
            def q_kproj(tc_idx):
                ts0 = tc_idx * QCH
                ps = psp.tile([P, 2 * QCH], F32, tag="sc")
                for dot in range(2):
                    col = slice(dot * QCH, (dot + 1) * QCH)
                    for dt in range(NDT):
                        mm_pair(
                            ps[:, col],
                            lambda h, dt=dt, dot=dot: wk_t[
                                :, dt, dot * P + h * 64:dot * P + (h + 1) * 64],
                            xt_k[:, dt, ts0:ts0 + QCH],
                            None,
                            start=(dt == 0), stop=(dt == NDT - 1))
                # per-head 64-row slices into the padded K^T; bias on DVE
                for h in range(HPC):
                    rows = slice((h % 2) * 64, (h % 2) * 64 + 64)
                    dot = h // 2
                    nc.vector.tensor_scalar_add(
                        kz_t[rows, h, ts0:ts0 + QCH],
                        ps[rows, dot * QCH:(dot + 1) * QCH],
                        bk_t[rows, dot, :])

            def q_qproj(tc_idx):
                ts0 = tc_idx * QCH
                ps = psp.tile([P, 2 * QCH], F32, tag="sc")
                for dot in range(2):
                    col = slice(dot * QCH, (dot + 1) * QCH)
                    for dt in range(NDT):
                        mm_pair(
                            ps[:, col],
                            lambda h, dt=dt, dot=dot: wq_t[
                                :, dt, dot * P + h * 64:dot * P + (h + 1) * 64],
                            xt_q[:, dt, ts0:ts0 + QCH],
                            None,
                            start=(dt == 0), stop=(dt == NDT - 1))
                for dot in range(2):
                    nc.vector.tensor_scalar_add(
                        qt_t[:, dot, ts0:ts0 + QCH],
                        ps[:, dot * QCH:(dot + 1) * QCH],
                        bq_t[:, dot, :])

            def q_vproj(tt):
                ps = psp.tile([P, 2 * QCH], F32, tag="sc")
                for dt in range(NDT):
                    mm_pair(
                        ps[:, 0:DS],
                        lambda h, dt=dt, tt=tt: xt_v[
                            :, dt, tt * P + h * 64:tt * P + (h + 1) * 64],
                        wv_t[:, dt, :],
                        None,
                        start=(dt == 0), stop=(dt == NDT - 1))
                nc.vector.tensor_add(v_t[:, tt, :], ps[:, 0:DS], bv_b[:, :])

            e_ring = {}

            def q_scores(s, kp):
                qc, pr = divmod(s, 2)
                qs = qc * QCH
                pair = []
                for hh in range(2):
                    hsel = 2 * pr + hh
                    sc = psp.tile([P, 2 * QCH], F32, tag="sc")
                    for j in range(2):
                        ks = (2 * kp + j) * P
                        col = slice(j * QCH, (j + 1) * QCH)
                        mm_pair(
                            sc[:, col],
                            lambda h, hsel=hsel, ks=ks: kz_t[
                                :, hsel, ks + h * 64:ks + (h + 1) * 64],
                            qt_t[:, pr, qs:qs + QCH],
                            None,
                            start=True, stop=True)
                    e = expp.tile([P, 2 * QCH], F16, tag="exp")
                    nc.scalar.activation(e[:], sc[:], AF.Exp, scale=SCALE)
                    pair.append(e)
                e_ring[(s, kp)] = pair

            acc_sm = {}

            def q_ph2(s, kp):
                qc, pr = divmod(s, 2)
                h0 = 2 * pr
                h1 = 2 * pr + 1
                if s not in acc_sm:
                    acc_sm[s] = (accp.tile([P, QCH], F32, tag="acc", name="acc"),
                                 smp.tile([P, QCH], F32, tag="sum", name="sm"))
                acc, sm = acc_sm[s]
                e0, e1 = e_ring.pop((s, kp))
                for j in range(2):
                    kt = 2 * kp + j
                    col = slice(j * QCH, (j + 1) * QCH)
                    st = (kt == 0)
                    sp = (kt == NKT - 1)
                    mm(sm[0:64, :], ones_t[:], e0[:, col],
                       start=st, stop=sp, tile_position=(0, 0))
                    mm(sm[64:128, :], ones_t[:], e1[:, col],
                       start=st, stop=sp, tile_position=(0, 64))
                    mm(acc[0:64, :], v_t[:, kt, h0 * DH:(h0 + 1) * DH],
                       e0[:, col], start=st, stop=sp, tile_position=(0, 0))
                    mm(acc[64:128, :], v_t[:, kt, h1 * DH:(h1 + 1) * DH],
                       e1[:, col], start=st, stop=sp, tile_position=(0, 64))

            def q_ph2_end(s):
                qc, pr = divmod(s, 2)
                qs = qc * QCH
                acc, sm = acc_sm.pop(s)
                rc = rcpp.tile([P, QCH], F32, tag="rcp")
                nc.vector.reciprocal_approx_fast(rc[:], sm[:])
                nc.vector.tensor_mul(an_t[:, pr, qs:qs + QCH], acc[:], rc[:])

            def q_outproj(qc, tt4):
                tt = qc * 4 + tt4
                po = psp.tile([P, 2 * QCH], F32, tag="sc")
                for half in range(2):
                    col = slice(half * QCH, (half + 1) * QCH)
                    for pr in range(2):
                        mm_pair(
                            po[:, col],
                            lambda h, pr=pr, tt=tt: an_t[
                                :, pr, tt * P + h * 64:tt * P + (h + 1) * 64],
                            wo_t[:, pr, half * QCH:(half + 1) * QCH],
                            None,
                            start=(pr == 0), stop=(pr == 1))
                ob = osbp.tile([P, D], F16, tag="osb")
                nc.vector.tensor_add(ob[:], po[:], bo_b[:])
                nc.sync.dma_start(
                    partial_cs[tt // 4][(tt % 4) * P:(tt % 4 + 1) * P, :],
                    ob[:])

            def q_rs(qc):
                nc.gpsimd.collective_compute(
                    "ReduceScatter",
                    mybir.AluOpType.add,
                    replica_groups=REPLICA_GROUPS,
                    ins=[partial_cs[qc][:]],
                    outs=[rs_cs[qc][:]],
                )
                nc.sync.dma_start(out_d[qc * P:(qc + 1) * P, :],
                                  rs_cs[qc][:])

            # ---- schedule ----
            # Global software pipeline over 64 (slot, kp) groups: iteration n
            # emits [attn@V of group n-2 | scores+exp of group n | fillers].
            # The lag-2 keeps exp ungated (its e-tile ring slot was freed 2
            # iterations ago) while finishing each slot's attn@V right after
            # its exps, so outproj(qc)+ReduceScatter land 2 slots later and
            # overlap the remaining compute instead of piling into the tail.
            for tci in range(NQC):
                q_kproj(tci)
            q_qproj(0)

            def fillers(n):
                s, kp = divmod(n, 8)
                if 1 <= n <= 8:
                    q_vproj(2 * (n - 1))
                    q_vproj(2 * (n - 1) + 1)
                if n == 14:
                    q_qproj(1)
                if n == 29:
                    q_qproj(2)
                if n == 45:
                    q_qproj(3)
                if s in (2, 4, 6) and kp in (3, 4, 6, 7):
                    qc = s // 2 - 1
                    q_outproj(qc, {3: 0, 4: 1, 6: 2, 7: 3}[kp])
                    if kp == 7:
                        q_rs(qc)

            for n in range(64):
                g = n - 2
                if g >= 0:
                    q_ph2(g // 8, g % 8)
                    if g % 8 == 7:
                        q_ph2_end(g // 8)
                q_scores(n // 8, n % 8)
                fillers(n)

            # tail
            q_ph2(7, 6)
            q_ph2(7, 7)
            q_ph2_end(7)
            for tt4 in range(4):
                q_outproj(3, tt4)
            q_rs(3)

    nc.compile()
    return nc


def _get_nc():
    global _CACHED_NC
    if _CACHED_NC is None:
        _CACHED_NC = _build_module()
    return _CACHED_NC


def _swap_pairs_rows(wo_slice):
    """Swap the two 64-row head blocks within each head pair (phase-2 PSUM
    layout has the pair's heads in partitions 0-63 / 64-127)."""
    out = wo_slice.copy()
    for pr in range(2):
        a = pr * 2 * DH
        out[a:a + DH], out[a + DH:a + 2 * DH] = \
            wo_slice[a + DH:a + 2 * DH].copy(), wo_slice[a:a + DH].copy()
    return out


def _make_in_maps(query, key, value, Wq, bq, Wk, bk, Wv, bv, Wo, bo):
    query = np.asarray(query, dtype=np.float32)
    key = np.asarray(key, dtype=np.float32)
    value = np.asarray(value, dtype=np.float32)
    Wq = np.asarray(Wq, dtype=np.float32)
    Wk = np.asarray(Wk, dtype=np.float32)
    Wv = np.asarray(Wv, dtype=np.float32)
    Wo = np.asarray(Wo, dtype=np.float32)
    bq = np.asarray(bq, dtype=np.float32)
    bk = np.asarray(bk, dtype=np.float32)
    bv = np.asarray(bv, dtype=np.float32)
    bo = np.asarray(bo, dtype=np.float32)

    in_maps = []
    for c in range(NCORES):
        b = c // GPB
        g = c % GPB
        sl = slice(g * DS, (g + 1) * DS)
        in_maps.append({
            "xq": query[b].astype(np.float16),
            "xk": key[b].astype(np.float16),
            "xv": value[b].astype(np.float16),
            "wq": Wq[:, sl].astype(np.float16),
            "wk": Wk[:, sl].astype(np.float16),
            "wv": Wv[:, sl].astype(np.float16),
            "wo": Wo[sl, :].astype(np.float16),
            "bq": bq[sl].reshape(DS, 1).copy(),
            "bk": bk[sl].reshape(DS, 1).copy(),
            "bv": bv[sl].reshape(1, DS).copy(),
            "bo": (bo if g == 0 else np.zeros_like(bo)).reshape(1, D).copy(),
        })
    return in_maps


def run(inputs, trace=False, trace_cores=None):
    """Run the SPMD kernel; returns (full_output, BassKernelResults)."""
    nc = _get_nc()
    in_maps = _make_in_maps(**inputs)
    res = run_bass_kernel_spmd(
        nc, in_maps, core_ids=list(range(NCORES)), trace=trace,
        trace_cores=trace_cores)
    out = np.empty((B, S, D), dtype=np.float32)
    for c in range(NCORES):
        b = c // GPB
        g = c % GPB
        o = res.results[c]["out"].astype(np.float32)
        for j in range(4):
            out[b, j * 512 + g * P:j * 512 + (g + 1) * P, :] = \
                o[j * P:(j + 1) * P, :]
    return out, res


def kernel(**inputs):
    out, _ = run(inputs, trace=False)
    return out


# revision 14
# speedup vs baseline: 1.3846x; 1.1017x over previous
"""Multi-head attention (B=2, S=2048, D=1024, H=16) on 8 Trainium2 NeuronCores.

Sharding: batch x head-group. Core c handles batch b = c//4 and heads
[4*(c%4), 4*(c%4)+4) (a 256-wide slice of the QKV projection output and the
matching 256-row slice of Wo). Each core computes its partial output
projection; a 4-way ReduceScatter per batch group sums the partials and
leaves each core with a [512, 1024] row block of the final output, which the
host reassembles.

v2 dataflow (all matmul operands fp16, fp32 PSUM accumulation):
  - x^T tiles via hardware DMA-transpose, token-split halves across BOTH
    HWDGE queues (SP + Activation) with cross-queue copy<->transpose
    ordering edges for the xbar-mode hazard.
  - EVERY matmul is a 64-out-row col-tiled pair (tile_position (0,0)/(0,64))
    in one uniform PE tiling mode: pairs dual-issue on the PE array halves
    (observed ~263ns per 512-col pair vs ~330ns for a plain 128-row matmul)
    and the constant mode means no array drains anywhere.
  - Attention is software-pipelined at (slot = (q-chunk, head-pair), kp =
    2-k-tile group) granularity: scores(s) | exp(s) on Scalar | attn@V+sums
    of slot s-1 | projection / output-projection fillers, sized so the
    Scalar engine's exp stream (the ~129us serial floor) stays saturated.
  - Softmax without max-subtraction; per-q sums via ones-matmuls col-packed
    with the attn@V pairs; normalization multiply on DVE; proj biases on DVE
    (tensor_scalar) keeping Scalar exp-only.
"""

import numpy as np

import concourse.bass as bass  # noqa: F401  (engine namespaces via nc)
import concourse.mybir as mybir
import concourse.tile as tile
from concourse import bacc
from concourse.bass import _add_dep_helper
from concourse.bass_utils import run_bass_kernel_spmd

F32 = mybir.dt.float32
F16 = mybir.dt.float16
AF = mybir.ActivationFunctionType

B, S, D = 2, 2048, 1024
H, DH = 16, 64
NCORES = 8
GPB = 4                # cores per batch group
HPC = H // GPB         # heads per core
DS = HPC * DH          # 256: per-core slice of the projection output
P = 128
NDT = D // P           # 8 d_model tiles
NTT = S // P           # 16 token tiles
QCH = 512              # q-chunk (PSUM bank = 512 fp32)
NQC = S // QCH         # 4
NKT = S // P           # 16 k tiles
HS = S // 2            # token-split half for the DMA transposes
SCALE = float(1.0 / np.sqrt(DH))

REPLICA_GROUPS = [[0, 1, 2, 3], [4, 5, 6, 7]]

_CACHED_NC = None


def _build_module():
    nc = bacc.Bacc("TRN2", target_bir_lowering=False, debug=False,
                   num_devices=NCORES)

    xq_d = nc.dram_tensor("xq", [S, D], F16, kind="ExternalInput")
    xk_d = nc.dram_tensor("xk", [S, D], F16, kind="ExternalInput")
    xv_d = nc.dram_tensor("xv", [S, D], F16, kind="ExternalInput")
    wq_d = nc.dram_tensor("wq", [D, DS], F16, kind="ExternalInput")
    wk_d = nc.dram_tensor("wk", [D, DS], F16, kind="ExternalInput")
    wv_d = nc.dram_tensor("wv", [D, DS], F16, kind="ExternalInput")
    wo_d = nc.dram_tensor("wo", [DS, D], F16, kind="ExternalInput")
    bq_d = nc.dram_tensor("bq", [DS, 1], F32, kind="ExternalInput")
    bk_d = nc.dram_tensor("bk", [DS, 1], F32, kind="ExternalInput")
    bv_d = nc.dram_tensor("bv", [1, DS], F32, kind="ExternalInput")
    bo_d = nc.dram_tensor("bo", [1, D], F32, kind="ExternalInput")

    out_d = nc.dram_tensor("out", [S // GPB, D], F16, kind="ExternalOutput")
    partial_cs = [nc.dram_tensor(f"partial{j}", [4 * P, D], F16)
                  for j in range(4)]
    rs_cs = [nc.dram_tensor(f"rs_out{j}", [P, D], F16)
             for j in range(4)]

    with tile.TileContext(nc) as tc:
        with (
            tc.tile_pool(name="cst", bufs=1) as cst,
            tc.tile_pool(name="exp", bufs=16) as expp,
            tc.tile_pool(name="rcp", bufs=2) as rcpp,
            tc.tile_pool(name="osb", bufs=3) as osbp,
            tc.tile_pool(name="ps", bufs=3, space="PSUM") as psp,
            tc.tile_pool(name="sm", bufs=1, space="PSUM") as smp,
            tc.tile_pool(name="acc", bufs=1, space="PSUM") as accp,
        ):
            # One uniform PE tiling mode (128-contraction, 64-out-row col
            # pairs) -> never a mode flip; nosync chain just pins emission
            # order so the interleave survives scheduling.
            _real_matmul = nc.tensor.matmul
            _prev_mm = {"inst": None}

            def mm(out, lhsT, rhs, **kw):
                inst = _real_matmul(out, lhsT, rhs, skip_group_check=True,
                                    **kw)
                if _prev_mm["inst"] is not None:
                    _add_dep_helper(inst.ins, _prev_mm["inst"].ins,
                                    sync=False, reason="pe-order")
                _prev_mm["inst"] = inst
                return inst

            def mm_pair(out, lhsT_ap, rhs, col0, **kw):
                """Two 64-row col-tiled matmuls writing out[0:64]/[64:128].

                lhsT_ap: callable half -> AP of [128, 64] weights for that
                output-row half; rhs shared.
                """
                mm(out[0:64, :], lhsT_ap(0), rhs,
                   tile_position=(0, 0), **kw)
                mm(out[64:128, :], lhsT_ap(1), rhs,
                   tile_position=(0, 64), **kw)

            # ---- constants ----
            wq_t = cst.tile([P, NDT, DS], F16, tag="wq")
            wk_t = cst.tile([P, NDT, DS], F16, tag="wk")
            wv_t = cst.tile([P, NDT, DS], F16, tag="wv")
            wo_t = cst.tile([P, 2, D], F16, tag="wo")
            bq_t = cst.tile([P, 2, 1], F32, tag="bq")
            bk_t = cst.tile([P, 2, 1], F32, tag="bk")
            bv_row = cst.tile([1, DS], F32, tag="bvr")
            bo_row = cst.tile([1, D], F32, tag="bor")
            bv_b = cst.tile([P, DS], F32, tag="bvb")
            bo_b = cst.tile([P, D], F32, tag="bob")
            ones_t = cst.tile([P, DH], F16, tag="ones")

            # ---- activations: resident tensors ----
            qt_t = cst.tile([P, 2, S], F16, tag="qt")    # Q^T  (pair, t)
            kz_t = cst.tile([P, HPC, S], F16, tag="kz")  # zero-padded K^T
            v_t = cst.tile([P, NTT, DS], F16, tag="vt")  # V token-major
            an_t = cst.tile([P, 2, S], F16, tag="an")    # attn_norm^T
            xt_k = cst.tile([P, NDT, S], F16, tag="xtk")
            xt_q = cst.tile([P, NDT, S], F16, tag="xtq")
            xt_v = cst.tile([P, NDT, S], F16, tag="xtv")

            # ---- DMA: one queue (SP), FIFO pinned with chain deps ----
            # The DMA xbar has a global transpose/copy mode and ~fixed
            # aggregate transpose throughput: concurrent queues just contend
            # (measured 0.65x each) and any copy in flight during transpose
            # mode corrupts (the v2 failure: the scheduler hoisted wv/wo
            # between transposes on the other queue). So: one queue, copies
            # strictly before transposes, order pinned.
            _prev_dma = {"inst": None}

            def dma(out, in_, **kw):
                inst = nc.sync.dma_start(out, in_, **kw)
                if _prev_dma["inst"] is not None:
                    _add_dep_helper(inst.ins, _prev_dma["inst"].ins,
                                    sync=False, reason="dma-fifo")
                _prev_dma["inst"] = inst
                return inst

            # Order tuned for earliest exp-start: K first (gates everything),
            # then the copies, then V's first quarter (slot-0 attn@V), Q's
            # first quarter (scores need only q-chunk s//2), rest of V, rest
            # of Q. Copies all precede the first needed point.
            dma(wk_t[:], wk_d.rearrange("(a p) n -> p a n", p=P))
            dma(bk_t[:], bk_d.rearrange("(a p) o -> p a o", p=P))
            dma(xt_k[:, :, 0:HS], xk_d[0:HS, :], transpose=True)
            dma(xt_k[:, :, HS:S], xk_d[HS:S, :], transpose=True)
            dma(bq_t[:], bq_d.rearrange("(a p) o -> p a o", p=P))
            dma(wq_t[:], wq_d.rearrange("(a p) n -> p a n", p=P))
            dma(wv_t[:], wv_d.rearrange("(a p) n -> p a n", p=P))
            dma(bv_row[:], bv_d[:])
            dma(xt_v[:, :, 0:QCH], xv_d[0:QCH, :], transpose=True)
            dma(xt_q[:, :, 0:QCH], xq_d[0:QCH, :], transpose=True)
            dma(xt_v[:, :, QCH:2 * QCH], xv_d[QCH:2 * QCH, :],
                transpose=True)
            dma(xt_v[:, :, 2 * QCH:3 * QCH], xv_d[2 * QCH:3 * QCH, :],
                transpose=True)
            dma(xt_v[:, :, 3 * QCH:S], xv_d[3 * QCH:S, :], transpose=True)
            dma(xt_q[:, :, QCH:2 * QCH], xq_d[QCH:2 * QCH, :],
                transpose=True)
            dma(wo_t[:], wo_d.rearrange("(a p) n -> p a n", p=P))
            dma(bo_row[:], bo_d[:])
            dma(xt_q[:, :, 2 * QCH:3 * QCH], xq_d[2 * QCH:3 * QCH, :],
                transpose=True)
            dma(xt_q[:, :, 3 * QCH:S], xq_d[3 * QCH:S, :], transpose=True)
            nc.gpsimd.partition_broadcast(bv_b[:], bv_row[:])
            nc.gpsimd.partition_broadcast(bo_b[:], bo_row[:])

            nc.vector.memset(kz_t[:], 0.0)
            nc.vector.memset(ones_t[:], 1.0)

            # ---- quanta ----
            def q_kproj(tc_idx):
                ts0 = tc_idx * QCH
                ps = psp.tile([P, 2 * QCH], F32, tag="sc")
                for dot in range(2):
                    col = slice(dot * QCH, (dot + 1) * QCH)
                    for dt in range(NDT):
                        mm_pair(
                            ps[:, col],
                            lambda h, dt=dt, dot=dot: wk_t[
                                :, dt, dot * P + h * 64:dot * P + (h + 1) * 64],
                            xt_k[:, dt, ts0:ts0 + QCH],
                            None,
                            start=(dt == 0), stop=(dt == NDT - 1))
                # per-head 64-row slices into the padded K^T; bias on DVE
                for h in range(HPC):
                    rows = slice((h % 2) * 64, (h % 2) * 64 + 64)
                    dot = h // 2
                    nc.vector.tensor_scalar_add(
                        kz_t[rows, h, ts0:ts0 + QCH],
                        ps[rows, dot * QCH:(dot + 1) * QCH],
                        bk_t[rows, dot, :])

            def q_qproj_dot(tc_idx, dot):
                ts0 = tc_idx * QCH
                ps = psp.tile([P, 2 * QCH], F32, tag="sc", name="psq")
                for dt in range(NDT):
                    mm_pair(
                        ps[:, 0:QCH],
                        lambda h, dt=dt, dot=dot: wq_t[
                            :, dt, dot * P + h * 64:dot * P + (h + 1) * 64],
                        xt_q[:, dt, ts0:ts0 + QCH],
                        None,
                        start=(dt == 0), stop=(dt == NDT - 1))
                nc.vector.tensor_scalar_add(
                    qt_t[:, dot, ts0:ts0 + QCH],
                    ps[:, 0:QCH],
                    bq_t[:, dot, :])

            def q_qproj(tc_idx):
                q_qproj_dot(tc_idx, 0)
                q_qproj_dot(tc_idx, 1)

            def q_vproj(tt):
                ps = psp.tile([P, 2 * QCH], F32, tag="sc")
                for dt in range(NDT):
                    mm_pair(
                        ps[:, 0:DS],
                        lambda h, dt=dt, tt=tt: xt_v[
                            :, dt, tt * P + h * 64:tt * P + (h + 1) * 64],
                        wv_t[:, dt, :],
                        None,
                        start=(dt == 0), stop=(dt == NDT - 1))
                nc.vector.tensor_add(v_t[:, tt, :], ps[:, 0:DS], bv_b[:, :])

            e_ring = {}

            def q_scores(s, kp):
                qc, pr = divmod(s, 2)
                qs = qc * QCH
                pair = []
                for hh in range(2):
                    hsel = 2 * pr + hh
                    sc = psp.tile([P, 2 * QCH], F32, tag="sc")
                    for j in range(2):
                        ks = (2 * kp + j) * P
                        col = slice(j * QCH, (j + 1) * QCH)
                        mm_pair(
                            sc[:, col],
                            lambda h, hsel=hsel, ks=ks: kz_t[
                                :, hsel, ks + h * 64:ks + (h + 1) * 64],
                            qt_t[:, pr, qs:qs + QCH],
                            None,
                            start=True, stop=True)
                    e = expp.tile([P, 2 * QCH], F16, tag="exp")
                    nc.scalar.activation(e[:], sc[:], AF.Exp, scale=SCALE)
                    pair.append(e)
                e_ring[(s, kp)] = pair

            acc_sm = {}

            def q_ph2(s, kp):
                qc, pr = divmod(s, 2)
                h0 = 2 * pr
                h1 = 2 * pr + 1
                if s not in acc_sm:
                    acc_sm[s] = (accp.tile([P, QCH], F32, tag="acc", name="acc"),
                                 smp.tile([P, QCH], F32, tag="sum", name="sm"))
                acc, sm = acc_sm[s]
                e0, e1 = e_ring.pop((s, kp))
                for j in range(2):
                    kt = 2 * kp + j
                    col = slice(j * QCH, (j + 1) * QCH)
                    st = (kt == 0)
                    sp = (kt == NKT - 1)
                    mm(sm[0:64, :], ones_t[:], e0[:, col],
                       start=st, stop=sp, tile_position=(0, 0))
                    mm(sm[64:128, :], ones_t[:], e1[:, col],
                       start=st, stop=sp, tile_position=(0, 64))
                    mm(acc[0:64, :], v_t[:, kt, h0 * DH:(h0 + 1) * DH],
                       e0[:, col], start=st, stop=sp, tile_position=(0, 0))
                    mm(acc[64:128, :], v_t[:, kt, h1 * DH:(h1 + 1) * DH],
                       e1[:, col], start=st, stop=sp, tile_position=(0, 64))

            def q_ph2_end(s):
                qc, pr = divmod(s, 2)
                qs = qc * QCH
                acc, sm = acc_sm.pop(s)
                rc = rcpp.tile([P, QCH], F32, tag="rcp")
                nc.vector.reciprocal_approx_fast(rc[:], sm[:])
                nc.vector.tensor_mul(an_t[:, pr, qs:qs + QCH], acc[:], rc[:])

            def q_outproj(qc, tt4):
                tt = qc * 4 + tt4
                po = psp.tile([P, 2 * QCH], F32, tag="sc")
                for half in range(2):
                    col = slice(half * QCH, (half + 1) * QCH)
                    for pr in range(2):
                        mm_pair(
                            po[:, col],
                            lambda h, pr=pr, tt=tt: an_t[
                                :, pr, tt * P + h * 64:tt * P + (h + 1) * 64],
                            wo_t[:, pr, half * QCH:(half + 1) * QCH],
                            None,
                            start=(pr == 0), stop=(pr == 1))
                ob = osbp.tile([P, D], F16, tag="osb")
                nc.vector.tensor_add(ob[:], po[:], bo_b[:])
                nc.sync.dma_start(
                    partial_cs[tt // 4][(tt % 4) * P:(tt % 4 + 1) * P, :],
                    ob[:])

            def q_rs(qc):
                # collective only; the out_d drains happen at the very end —
                # an out_d DMA emitted here would head-of-line-block the SP
                # queue behind the collective, wedging later partial writes
                # and (through the osb/psum pool rings) the whole machine.
                nc.gpsimd.collective_compute(
                    "ReduceScatter",
                    mybir.AluOpType.add,
                    replica_groups=REPLICA_GROUPS,
                    ins=[partial_cs[qc][:]],
                    outs=[rs_cs[qc][:]],
                )

            # ---- schedule ----
            # Global software pipeline over 64 (slot, kp) groups: iteration n
            # emits [attn@V of group n-2 | scores+exp of group n | fillers].
            # The lag-2 keeps exp ungated (its e-tile ring slot was freed 2
            # iterations ago) while finishing each slot's attn@V right after
            # its exps, so outproj(qc)+ReduceScatter land 2 slots later and
            # overlap the remaining compute instead of piling into the tail.
            # PE pstate warmup: keep the array continuously busy from ~3us in
            # so the K projection runs at full clock the moment x^T lands
            # (the ramp needs ~3us of continuous execution; a cold K proj
            # costs ~2x).
            pw = psp.tile([P, 2 * QCH], F32, tag="sc", name="warm")
            for _ in range(150):
                mm(pw[0:64, 0:DH], ones_t[:], ones_t[:],
                   start=True, stop=True, tile_position=(0, 0))
                mm(pw[64:128, 0:DH], ones_t[:], ones_t[:],
                   start=True, stop=True, tile_position=(0, 64))

            for tci in range(NQC):
                q_kproj(tci)
            q_qproj(0)

            def fillers(n):
                s, kp = divmod(n, 8)
                if 1 <= n <= 8:
                    q_vproj(2 * (n - 1))
                    q_vproj(2 * (n - 1) + 1)
                if n in (14, 15):
                    q_qproj_dot(1, n - 14)
                if n in (29, 30):
                    q_qproj_dot(2, n - 29)
                if n in (45, 46):
                    q_qproj_dot(3, n - 45)
                if s in (2, 4, 6) and kp in (3, 4, 6, 7):
                    qc = s // 2 - 1
                    q_outproj(qc, {3: 0, 4: 1, 6: 2, 7: 3}[kp])
                    if kp == 7:
                        q_rs(qc)

            for n in range(64):
                g = n - 2
                if g >= 0:
                    q_ph2(g // 8, g % 8)
                    if g % 8 == 7:
                        q_ph2_end(g // 8)
                q_scores(n // 8, n % 8)
                fillers(n)

            # tail
            q_ph2(7, 6)
            q_ph2(7, 7)
            q_ph2_end(7)
            for tt4 in range(4):
                q_outproj(3, tt4)
            q_rs(3)
            for qc in range(4):
                nc.sync.dma_start(out_d[qc * P:(qc + 1) * P, :],
                                  rs_cs[qc][:])

    nc.compile()
    return nc


def _get_nc():
    global _CACHED_NC
    if _CACHED_NC is None:
        _CACHED_NC = _build_module()
    return _CACHED_NC


def _swap_pairs_rows(wo_slice):
    """Swap the two 64-row head blocks within each head pair (phase-2 PSUM
    layout has the pair's heads in partitions 0-63 / 64-127)."""
    out = wo_slice.copy()
    for pr in range(2):
        a = pr * 2 * DH
        out[a:a + DH], out[a + DH:a + 2 * DH] = \
            wo_slice[a + DH:a + 2 * DH].copy(), wo_slice[a:a + DH].copy()
    return out


def _make_in_maps(query, key, value, Wq, bq, Wk, bk, Wv, bv, Wo, bo):
    query = np.asarray(query, dtype=np.float32)
    key = np.asarray(key, dtype=np.float32)
    value = np.asarray(value, dtype=np.float32)
    Wq = np.asarray(Wq, dtype=np.float32)
    Wk = np.asarray(Wk, dtype=np.float32)
    Wv = np.asarray(Wv, dtype=np.float32)
    Wo = np.asarray(Wo, dtype=np.float32)
    bq = np.asarray(bq, dtype=np.float32)
    bk = np.asarray(bk, dtype=np.float32)
    bv = np.asarray(bv, dtype=np.float32)
    bo = np.asarray(bo, dtype=np.float32)

    in_maps = []
    for c in range(NCORES):
        b = c // GPB
        g = c % GPB
        sl = slice(g * DS, (g + 1) * DS)
        in_maps.append({
            "xq": query[b].astype(np.float16),
            "xk": key[b].astype(np.float16),
            "xv": value[b].astype(np.float16),
            "wq": Wq[:, sl].astype(np.float16),
            "wk": Wk[:, sl].astype(np.float16),
            "wv": Wv[:, sl].astype(np.float16),
            "wo": Wo[sl, :].astype(np.float16),
            "bq": bq[sl].reshape(DS, 1).copy(),
            "bk": bk[sl].reshape(DS, 1).copy(),
            "bv": bv[sl].reshape(1, DS).copy(),
            "bo": (bo if g == 0 else np.zeros_like(bo)).reshape(1, D).copy(),
        })
    return in_maps


def run(inputs, trace=False, trace_cores=None):
    """Run the SPMD kernel; returns (full_output, BassKernelResults)."""
    nc = _get_nc()
    in_maps = _make_in_maps(**inputs)
    res = run_bass_kernel_spmd(
        nc, in_maps, core_ids=list(range(NCORES)), trace=trace,
        trace_cores=trace_cores)
    out = np.empty((B, S, D), dtype=np.float32)
    for c in range(NCORES):
        b = c // GPB
        g = c % GPB
        o = res.results[c]["out"].astype(np.float32)
        for j in range(4):
            out[b, j * 512 + g * P:j * 512 + (g + 1) * P, :] = \
                o[j * P:(j + 1) * P, :]
    return out, res


def kernel(**inputs):
    out, _ = run(inputs, trace=False)
    return out
